# revision 55
# baseline (speedup 1.0000x reference)
"""Trainium2 Bass kernel: TextCNN (conv k=3/4/5 over [B,1,S,E] + relu +
global max-pool + FC + log_softmax), data-parallel over batch on 8 cores.

Conv runs as fp8(e4m3) DoubleRow matmuls: each matmul carries two 128-row
K-slots (e-chunks 0:128 and 128:256 of the E=300 contraction), so a full
tap is one matmul; the 44-row e-tails of all taps of a branch are stacked
into one extra DoubleRow matmul per branch from host-baked pre-shifted
copies. 15 matmuls per 4-batch group (the minimum for this shape) vs 30
for the fp32r formulation, streaming at 1 PSUM column/cycle. Host bakes
x*16 and w*512 into e4m3 (measured 1.23e-2 rel err vs the 2e-2 gate);
the 2^-13 descale rides the relu activation's scale operand. x streams
per-group s-major (batch-innermost) so matmul ifmap APs stay 3-dim (the
DoubleRow slot strides are 16-byte aligned per the s3_lw ISA rule); the
max-pool reduce reads PSUM through a transposed AP. Each group's x loads
as one combined 128x4128B DMA (4 row-stripes for groups 0/1 for fast
arrival, prefetched 2 groups ahead after that); fp32r junk matmuls
bridge the first DMA so the PE HAM clock never drops. Every instruction
carries <=1 semaphore wait (single TPB wait slot): chained dummy-matmul
fence ticks absorb DMA/psum-WAR waits, and their sync=False deps pin the
tile scheduler's order so wait elision holds; the FC is deferred two
groups so the reduce->relu chain never stalls the PE. The kernel-tail
drain is split per semaphore proc.

Self-contained: hardcodes shapes/sharding; only imports the container
toolchain at /opt/trn_rl_repo.
"""

import sys

import ml_dtypes
import numpy as np

sys.path.insert(0, "/opt/trn_rl_repo")

import concourse.bass as bass  # noqa: E402
import concourse.tile as tile  # noqa: E402
from concourse import mybir  # noqa: E402
from concourse.bass_utils import run_bass_kernel_spmd  # noqa: E402
from concourse.tile import add_dep_helper  # noqa: E402
from concourse.vector_clock import ScopedClock, VectorClock  # noqa: E402

B, S, E = 512, 128, 300
NF = 100
NCLS = 5
NCORES = 8
BPC = B // NCORES  # 64 batch elems per core
G = 4  # batch elems per matmul group (4*128 = 512 psum cols)
NG = BPC // G  # 16 groups
PAD = 2
SP = S + 2 * PAD  # 132 padded seq length
KS = (3, 4, 5)
SOUT = {3: S - 2, 4: S - 1, 5: S}  # valid conv output positions
SMM = {3: S - 2, 4: S, 5: S}  # matmul cols per batch elem
JBASE = {3: 0, 4: 3, 5: 7}  # tap index base into the main weight tile
E2, E2N = 256, 44  # e-tail start / length
KTAIL = {3: 66, 4: 88, 5: 110}  # tail-stack partitions per branch

XSCALE = 16.0
WSCALE = 512.0
DESCALE = 1.0 / (XSCALE * WSCALE)

# tail stacking: (slot, row0, tap, e0, e1) segments per branch; both the
# x bake and the w bake follow this map so lhsT/rhs rows line up
TSTACK = {
    3: ((0, 0, 0, 0, 44), (0, 44, 1, 0, 22), (1, 0, 1, 22, 44), (1, 22, 2, 0, 44)),
    4: ((0, 0, 0, 0, 44), (0, 44, 1, 0, 44), (1, 0, 2, 0, 44), (1, 44, 3, 0, 44)),
    5: (
        (0, 0, 0, 0, 44),
        (0, 44, 1, 0, 44),
        (0, 88, 2, 0, 22),
        (1, 0, 2, 22, 44),
        (1, 22, 3, 0, 44),
        (1, 66, 4, 0, 44),
    ),
}

# DRAM row layout of the fp8 blob
WROW = 128
ROWS = WROW + 128
# tail x blocks padded to 512 so the DoubleRow slot stride is 16-aligned
TBLK = 512
# per-group combined block: xm | t3 | t4 | t5 side by side in one
# 128-row region so each group loads as 4 row-stripe DMAs on 4 queues
XMW = 2 * SP * G  # 1056
TOFF = {3: XMW, 4: XMW + 2 * TBLK, 5: XMW + 4 * TBLK}
GBLK = XMW + 6 * TBLK  # 4128
MW = NG * GBLK  # 66048
NSTRIPE = 4
# weight slot stride padded 100 -> 112 for the ISA 16-align rule
WPAD = 112
WMW = 12 * 2 * WPAD  # 2688
WTCOL = {3: WMW, 4: WMW + 2 * WPAD, 5: WMW + 4 * WPAD}

AUXW = 3 + 3 * NCLS

_f8 = mybir.dt.float8e4
_f32 = mybir.dt.float32
_f32r = mybir.dt.float32r

_built = None


def _off(k, t):
    return 5 - k + t


def _ins(i):
    return i.ins if hasattr(i, "ins") else i


def _dep(from_inst, to_inst, reason, sync=True):
    add_dep_helper(_ins(from_inst), _ins(to_inst), sync=sync, reason=reason)


class _SplitDrainTC(tile.TileContext):
    """TileContext whose kernel-tail drain is split into one drain per
    semaphore proc: the stock single drain carries one wait per used proc,
    which overflows the CTRL_NO encoding's wait slots on this toolchain."""

    def _drain_and_barrier(self, tick_clock, wait_clock):
        gc = tick_clock.global_clock
        ticks = eval(str(gc).replace("VectorClock", ""))
        for idx, tick in enumerate(ticks):
            if tick > 0:
                sub = VectorClock()
                sub.require_at_least(idx, tick)
                d = self.nc.sync.drain()
                wait_clock.add_sem_waits(d.ins, ScopedClock({None: sub}))
        self.nc.all_engine_barrier()
        assert self.sems is not None
        popped = self.nc._tile_sem_poison_stack.pop()
        assert popped is self._sem_poison
        self.nc.clear_and_free_semaphores(list(self.sems.allocated().values()))
        self.nc.all_engine_barrier()


def _build():
    nc = bass.Bass()
    xq = nc.declare_dram_parameter("xq", [ROWS, MW], _f8, isOutput=False)
    aux = nc.declare_dram_parameter("aux", [NF + 1, AUXW], _f32, isOutput=False)
    out = nc.declare_dram_parameter("out", [NCLS, BPC], _f32, isOutput=True)

    act = mybir.ActivationFunctionType
    DR = mybir.MatmulPerfMode.DoubleRow

    with _SplitDrainTC(nc) as tc:
        with (
            tc.tile_pool(name="consts", bufs=1) as consts,
            tc.tile_pool(name="xin", bufs=16) as xin,
            tc.tile_pool(name="small", bufs=4) as small,
            tc.tile_pool(name="feat", bufs=1) as featp,
            tc.tile_pool(name="psum", bufs=2, space="PSUM") as psum,
            tc.tile_pool(name="psfc", bufs=1, space="PSUM") as psfc,
        ):
            pescr = psfc.tile([128, 512], _f32, tag="pescr")
            dscr = small.tile([1, 2], _f32, tag="dscr")
            nc.vector.memset(dscr[:], 0.5)

            # fence ticks: dummy 1x1 PE matmuls, each carrying one sem
            # wait, so real matmuls stay within the single TPB wait slot
            def _tick():
                return nc.tensor.matmul(
                    pescr[0:1, 0:1],
                    dscr[0:1, 0:1],
                    dscr[0:1, 1:2],
                    start=True,
                    stop=True,
                )

            xtiles = {}

            def _xviews(xg, base):
                xm = xg[:, base : base + XMW].rearrange(
                    "p (two n) -> p two n", two=2
                )
                tails = {
                    k: xg[
                        : KTAIL[k], base + TOFF[k] : base + TOFF[k] + 2 * TBLK
                    ].rearrange("p (two n) -> p two n", two=2)
                    for k in KS
                }
                return xm, tails

            def make_x(g):
                # groups 0/1 stripe across 4 queues for fast arrival;
                # later groups (prefetched 2 ahead) load as one DMA so the
                # consuming matmul's auto-dep is a single wait
                if g in xtiles:
                    return xtiles[g]
                xg = xin.tile([128, GBLK], _f8, tag="xg", name=f"xg_{g}")
                ns = NSTRIPE if g < 2 else 1
                rs = 128 // ns
                ds = [
                    nc.sync.dma_start(
                        out=xg[q * rs : (q + 1) * rs, :],
                        in_=xq[q * rs : (q + 1) * rs, g * GBLK : (g + 1) * GBLK],
                    )
                    for q in range(ns)
                ]
                xtiles[g] = (_xviews(xg, 0), ds)
                return xtiles[g]

            # prewarm: full-array fp32r dummy matmuls bridge the DMA ramp
            # so the HAM clock gate is at 8/8 when the real matmuls start
            junkf = small.tile([128, 512], _f32, tag="junkf")
            nc.vector.memset(junkf[:], 0.25)
            junk = small.tile([128, 512], _f32r, tag="junk")
            nc.vector.tensor_copy(junk[:], junkf[:])
            for _ in range(6):
                nc.tensor.matmul(
                    pescr[:, :],
                    junk[:, :128],
                    junk[:, :],
                    start=True,
                    stop=True,
                )

            make_x(0)

            wdmas = {}
            wm = consts.tile([128, 12, 2, WPAD], _f8, tag="wm", name="wm")
            wdmas["wm"] = nc.sync.dma_start(
                out=wm[:],
                in_=xq[WROW : WROW + 128, :WMW].rearrange(
                    "p (j two m) -> p j two m", j=12, two=2
                ),
            )
            wt = {}
            for k in KS:
                t = consts.tile([KTAIL[k], 2, WPAD], _f8, tag=f"wt{k}", name=f"wt{k}")
                wdmas[f"wt{k}"] = nc.sync.dma_start(
                    out=t[:],
                    in_=xq[
                        WROW : WROW + KTAIL[k], WTCOL[k] : WTCOL[k] + 2 * WPAD
                    ].rearrange("p (two m) -> p two m", two=2),
                )
                wt[k] = t
            auxt = consts.tile([NF + 1, AUXW], _f32, tag="aux", name="aux")
            aux_dma = nc.sync.dma_start(out=auxt[:], in_=aux[:, :])
            make_x(1)

            ascratch = small.tile([1, 1], _f32, tag="ascratch")

            feats = [
                featp.tile([NF, BPC], _f32, tag=f"feat{kk}", name=f"feat{kk}")
                for kk in range(3)
            ]
            featr = [
                featp.tile(
                    [NF + (1 if kk == 2 else 0), BPC],
                    _f32,
                    tag=f"featr{kk}",
                    name=f"featr{kk}",
                )
                for kk in range(3)
            ]
            nc.vector.memset(featr[2][:], 1.0)

            plT = psfc.tile([NCLS, BPC], _f32, tag="plT")
            ones5 = small.tile([NCLS, 1], _f32, tag="ones5")
            nc.vector.memset(ones5[:], 1.0)
            mones1 = small.tile([1, NCLS], _f32, tag="mones1")
            nc.vector.memset(mones1[:], -1.0)
            afence = nc.scalar.memzero(ascratch[:])
            _dep(afence, aux_dma, "act waits aux")
            # touch Exp/Ln tables now so the tail doesn't pay cold loads
            nc.scalar.activation(ascratch[:], ascratch[:], act.Exp)
            nc.scalar.activation(ascratch[:], ascratch[:], act.Ln)

            def emit_fc(s):
                pend = None
                if s == 0:
                    pend = _tick()
                    _dep(pend, aux_dma, "aux loaded for FC")
                for kk in range(3):
                    krows = NF + (1 if kk == 2 else 0)
                    wsl = auxt[:krows, 3 + NCLS * kk : 3 + NCLS * (kk + 1)]
                    m = nc.tensor.matmul(
                        plT[:, 16 * s : 16 * (s + 1)],
                        wsl,
                        featr[kk][:krows, 16 * s : 16 * (s + 1)],
                        start=(s == 0 and kk == 0),
                        stop=(s == 3 and kk == 2),
                    )
                    if pend is not None:
                        _dep(m, pend, "fc aux fence", sync=False)
                        pend = None

            reds = {}
            for g in range(NG):
                (xm, tails), xdmas = make_x(g)
                if g + 2 < NG:
                    make_x(g + 2)  # prefetch: 1-stripe DMA needs lead time

                # PE-tick fence chain: each tick absorbs one sem wait;
                # the sync=False chain pins scheduler order so the waits
                # are elided from the real matmuls (which then carry at
                # most the framework's own psum-drain PE wait)
                fence = None

                def _chain(nop, fence):
                    if fence is not None:
                        _dep(nop, fence, "chain", sync=False)
                    return nop

                if g == 0:
                    nop = _tick()
                    _dep(nop, wdmas["wm"], "wm loaded")
                    fence = _chain(nop, fence)
                for xd in xdmas:
                    nop = _tick()
                    _dep(nop, xd, "x loaded")
                    fence = _chain(nop, fence)
                if g >= 2:
                    # absorbs the psum-bank WAR (reduces of g-2, vector)
                    nop = _tick()
                    for r in reds[g - 2]:
                        _dep(nop, r, "psum released")
                    fence = _chain(nop, fence)

                # FC for superblock s runs two groups after its last relu
                # so the reduce->relu chain has drained; it also fills the
                # PE while the group-start fence wait settles
                if g >= 6 and (g - 6) % 4 == 0:
                    emit_fc((g - 6) // 4)

                reds[g] = []
                for kk, k in enumerate(KS):
                    smm = SMM[k]
                    ps = psum.tile([NF, smm, G], _f32, tag=f"y{k}", name=f"y{k}_{g}")

                    pend = [fence]
                    n = 0

                    def mm_step(lhsT, rhs):
                        nonlocal n
                        m = nc.tensor.matmul(
                            ps[:],
                            lhsT,
                            rhs,
                            start=(n == 0),
                            stop=(n == k),
                            perf_mode=DR,
                        )
                        if pend[0] is not None:
                            _dep(m, pend[0], "fence", sync=False)
                            pend[0] = None
                        n += 1
                        return m

                    for t in range(k):
                        o = _off(k, t) * G
                        mm_step(
                            wm[:, JBASE[k] + t, :, :NF],
                            xm[:, :, o : o + smm * G],
                        )
                    if g == 0:
                        # first group: tail weights land after the mains
                        nop = _tick()
                        _dep(nop, wdmas[f"wt{k}"], f"wt{k} loaded")
                        pend[0] = nop
                    mm = mm_step(
                        wt[k][:, :, :NF], tails[k][:, :, : smm * G]
                    )
                    red = nc.vector.reduce_max(
                        feats[kk][:, g * G : (g + 1) * G],
                        ps[:, : SOUT[k], :].rearrange("p s g -> p g s"),
                        axis=mybir.AxisListType.X,
                    )
                    reds[g].append(red)
                    r = nc.scalar.activation(
                        featr[kk][:NF, g * G : (g + 1) * G],
                        feats[kk][:, g * G : (g + 1) * G],
                        act.Relu,
                        bias=auxt[:NF, kk : kk + 1],
                        scale=DESCALE,
                    )
                    _dep(r, afence, "act fence", sync=False)

            emit_fc(3)

            # log_softmax in transposed layout: x - ln(sum exp x), the
            # class-dim reduction and broadcast both done with tiny matmuls
            expT = small.tile([NCLS, BPC], _f32, tag="expT")
            nc.scalar.activation(expT[:], plT[:], act.Exp)
            nc.tensor.matmul(
                pescr[0:1, 64:128], ones5[:], expT[:], start=True, stop=True
            )
            lns = small.tile([1, BPC], _f32, tag="lns")
            nc.scalar.activation(lns[:], pescr[0:1, 64:128], act.Ln)
            nc.tensor.matmul(
                plT[:], mones1[:], lns[:], start=False, stop=True,
                skip_group_check=True,
            )
            ot = small.tile([NCLS, BPC], _f32, tag="ot")
            nc.vector.tensor_copy(ot[:], plT[:])
            nc.gpsimd.dma_start(out=out[:, :], in_=ot[:])
    return nc


def _q8(a, scale):
    return np.asarray(np.asarray(a, np.float32) * scale, ml_dtypes.float8_e4m3)


def _prep(x, w3, b3, w4, b4, w5, b5, Wfc, bfc):
    x = np.asarray(x, dtype=np.float32).reshape(B, S, E)
    ws = {3: np.asarray(w3, np.float32)[:, 0], 4: np.asarray(w4, np.float32)[:, 0],
          5: np.asarray(w5, np.float32)[:, 0]}
    w8 = {k: _q8(ws[k], WSCALE) for k in KS}  # [NF, k, E]

    base = np.zeros((ROWS, MW), ml_dtypes.float8_e4m3)
    # main weights [128, 12, 2, WPAD]: tap j, slot = e-chunk
    wmv = base[WROW : WROW + 128, :WMW].reshape(128, 12, 2, WPAD)
    for k in KS:
        for t in range(k):
            blk = w8[k][:, t, :].T  # [E, NF]
            wmv[:, JBASE[k] + t, 0, :NF] = blk[0:128]
            wmv[:, JBASE[k] + t, 1, :NF] = blk[128:256]
    # tail weights [KTAIL, 2, WPAD] following TSTACK
    for k in KS:
        wtv = base[WROW : WROW + KTAIL[k], WTCOL[k] : WTCOL[k] + 2 * WPAD].reshape(
            KTAIL[k], 2, WPAD
        )
        for sl, r0, t, e0, e1 in TSTACK[k]:
            wtv[r0 : r0 + (e1 - e0), sl, :NF] = w8[k][:, t, E2 + e0 : E2 + e1].T

    auxa = np.zeros((NF + 1, AUXW), np.float32)
    for kk, bb in enumerate((b3, b4, b5)):
        auxa[:NF, kk] = np.asarray(bb, np.float32)
    Wfc = np.asarray(Wfc, np.float32)
    for kk in range(3):
        auxa[:NF, 3 + NCLS * kk : 3 + NCLS * (kk + 1)] = Wfc[
            :, kk * NF : (kk + 1) * NF
        ].T
    auxa[NF, 3 + 2 * NCLS : 3 + 3 * NCLS] = np.asarray(bfc, np.float32)

    # x: [E, B, SP] padded, quantized once for all cores
    xt_all = np.zeros((E, B, SP), ml_dtypes.float8_e4m3)
    xt_all[:, :, PAD : PAD + S] = _q8(x.transpose(2, 0, 1), XSCALE)
    shards = []
    for c in range(NCORES):
        arr = base.copy()
        xc = xt_all[:, c * BPC : (c + 1) * BPC, :]  # [300, 64, 132]
        v = arr[0:128, :MW].reshape(128, NG, GBLK)
        # main block [128, NG, 2, SP, G] (s-major, batch-inner)
        m = np.stack((xc[0:128], xc[128:256]), axis=1)  # [128, 2, 64, 132]
        m = np.ascontiguousarray(
            m.reshape(128, 2, NG, G, SP).transpose(0, 2, 1, 4, 3)
        )
        v[:, :, :XMW] = m.reshape(128, NG, XMW)
        # tail blocks [KTAIL, NG, 2, TBLK] with baked shifts; first
        # smm*G cols of each slot used, rest zero padding
        tail = xc[E2:]  # [44, 64, 132]
        for k in KS:
            smm = SMM[k]
            for sl, r0, t, e0, e1 in TSTACK[k]:
                o = _off(k, t)
                ne = e1 - e0
                seg = tail[e0:e1, :, o : o + smm]  # [ne, 64, smm]
                v[
                    r0 : r0 + ne,
                    :,
                    TOFF[k] + sl * TBLK : TOFF[k] + sl * TBLK + smm * G,
                ] = (
                    seg.reshape(ne, NG, G, smm)
                    .transpose(0, 1, 3, 2)
                    .reshape(ne, NG, smm * G)
                )
        shards.append(arr)
    return shards, auxa


def _run(inputs, **spmd_kwargs):
    global _built
    if _built is None:
        _built = _build()
    shards, auxa = _prep(**inputs)
    in_maps = [{"xq": shards[c], "aux": auxa} for c in range(NCORES)]
    res = run_bass_kernel_spmd(_built, in_maps, list(range(NCORES)), **spmd_kwargs)
    outp = np.concatenate(
        [np.asarray(res.results[c]["out"]).T for c in range(NCORES)], axis=0
    )
    return outp, res


def kernel(**inputs):
    outp, _ = _run(inputs)
    return outp
